# revision 22
# baseline (speedup 1.0000x reference)
"""Trainium2 Bass kernel for DelayedAgg GNN message passing.

Per batch b (one NeuronCore per batch, B=8 across 8 cores):
    xin  = concat(features[b], support_xyz[b].T, ones)    # [68, N] bf16
    x1   = relu(W1' @ xin)                                # [128, N] (b1 folded
                                                          #  into W1' col 67)
    x2nb = W2 @ x1                                        # [256, N] (NO bias)
    out[c, m] = relu(max_k x2nb[c, idx[m,k]] + b2[c])
bias2 + relu2 commute out of the neighbor max, so the device computes only
max_k of bias-free conv2 rows; +b2/relu/transpose run on the host over the
small [6016, 256] result.

Measured 947-1058us baseline -> 573us on this kernel. Changes that got it:
  - All conv matmuls in bf16 (PE fp32 runs at 1/4 rate; fp32 convs made
    phase 1 ~330us wall; bf16 phase 1 is ~100us).  b1 folded into the matmul
    via a constant ones row (activation bias can't vary along the free dim
    in the transposed conv).  rel_err 0.002 -> 0.0049, still << 2e-2.
  - conv2 pairs two 128-point chunks into one [128, 512] PSUM tile so the
    PSUM->SBUF bf16 casts are half as many, alternating DVE/ACT; conv1
    relu+cast alternates ACT activation / DVE tensor_scalar_max.
  - dst pool bufs 4 -> 8 (phase-1 pools released first) removes
    gather-to-tree buffer-reuse stalls: descgen drops from 14.4us to
    ~10us per 4096-row call (the baseline's extra 4us was WAW sem waits
    inside the gather's engine slot, not ucode time).
  - 3MB idx table DMA moved to the ACT HWDGE queue so the first conv
    supertile's xin load isn't queued behind it on sync (~12us).
  - last supertile's x2t write split across both HWDGE queues (it gates
    every gather); last m-tile gathered as 4x 1024-idx sub-calls across
    all 4 queues so the final 2MB drain parallelizes instead of trailing
    one queue.
  - NEVER make a matmul output PSUM tile span >1 bank (2KB/part): a
    [128, 1024] f32 ps2 tile put the PE on a ~100x slow path (7.8ms).
Phase 2 is descgen-bound: InstDMAGatherAnt descriptor generation runs on one
Q7 core pair at ~2.5ns/row-descriptor and each call occupies the whole gpsimd
engine (queue_num only picks the SDMA ring), so 192k gathered rows floor at
~460-490us.  Dead ends verified on HW this session: prepare_only descgen
cannot hide under phase 1 (SWDGE ring carveout ~1024 desc/queue, one 4096-row
prep overflows it and deadlocks -> NRT INTERNAL error); indirect_dma_start
unrolls to per-index DMAs (249us per 4096 rows, and wrong layout); SBUF-source
and DRAM transpose-mode gathers (256B rows) measure no faster than the plain
512B gather (descgen is the wall, not SDMA); plain 256B-row gather is 4x
SLOWER (59.9us/call anomaly); activity-throttle (50% duty, active ~90% of
phase 2) does NOT slow the Q7 descgen path measurably.
Pitfalls baked in: single_packet=False (HW caps packets at ~64 descriptors);
cce_op=max rejected (CCE only does add).
"""

import os
import sys

import numpy as np

try:
    import concourse.bass as bass  # noqa: F401
except ImportError:  # pragma: no cover - container default path
    sys.path.insert(0, "/opt/trn_rl_repo")

import concourse.bass as bass
import concourse.bacc as bacc
import concourse.tile as tile
from concourse import mybir
from concourse.bass_utils import run_bass_kernel_spmd

import ml_dtypes

# Problem shapes (hardcoded per spec nn_DelayedAgg_76690936037739)
B = 8
N = 24000
M = 6000
K = 32
CIN = 64
CMID = 128
COUT = 256
CXIN = CIN + 3 + 1    # features + xyz + ones row (bias fold)

SUP = 2048            # support points per phase-1 supertile
NPAD = 24576          # 12 * 2048 = 192 * 128
NSUP = NPAD // SUP    # 12
MT = 47               # m-tiles of 128 queries
MPAD = MT * 128       # 6016
STG = 8               # m-tiles per output staging buffer
T = 128               # queries per gather call
NI = K * T            # 4096 indices per call
NW = NI // 16         # idx int16 words per partition per call
NQ = 4                # SWDGE queues for gather descgen

FP32 = mybir.dt.float32
BF16 = mybir.dt.bfloat16

_CACHE = {}


def build_body(ctx, tc, xin, w1t, w2t, idxt, outT, x2t):
    nc = tc.nc

    singles = ctx.enter_context(tc.tile_pool(name="singles", bufs=1))
    w1t_sb = singles.tile([CXIN, CMID], BF16)
    nc.sync.dma_start(out=w1t_sb[:], in_=w1t.ap())
    w2t_sb = singles.tile([CMID, COUT], BF16)
    nc.sync.dma_start(out=w2t_sb[:], in_=w2t.ap())
    # 3MB idx load goes on the gpsimd SWDGE queue: that engine is idle all
    # of phase 1 (and is the eventual consumer), so neither the sync-queue
    # xin loads nor the ACT-queue relus ever queue behind it.
    idx_sb = singles.tile([128, MT * NW], mybir.dt.int16)
    nc.gpsimd.dma_start(out=idx_sb[:], in_=idxt.ap())

    from contextlib import ExitStack

    with ExitStack() as p1:
        xin_pool = p1.enter_context(tc.tile_pool(name="xin", bufs=3))
        ps1_pool = p1.enter_context(tc.tile_pool(name="ps1", bufs=2, space="PSUM"))
        x1_pool = p1.enter_context(tc.tile_pool(name="x1", bufs=3))
        ps2_pool = p1.enter_context(tc.tile_pool(name="ps2", bufs=6, space="PSUM"))
        stage_pool = p1.enter_context(tc.tile_pool(name="stage", bufs=3))

        xin_ap = xin.ap()                                   # [68, NPAD]
        # x2t row (i*16 + j)*128 + p  <-  stage[p, j, :] for supertile i
        x2t_v = x2t.ap().rearrange("(i t p) c -> i p t c", t=SUP // 128, p=128)

        relu = mybir.ActivationFunctionType.Relu
        mx0 = mybir.AluOpType.max
        for i in range(NSUP):
            xin_sb = xin_pool.tile([CXIN, SUP], BF16)
            nc.sync.dma_start(out=xin_sb[:], in_=xin_ap[:, i * SUP:(i + 1) * SUP])
            x1_sb = x1_pool.tile([CMID, SUP], BF16)
            for q in range(SUP // 512):
                ps1 = ps1_pool.tile([CMID, 512], FP32)
                nc.tensor.matmul(
                    ps1[:], lhsT=w1t_sb[:], rhs=xin_sb[:, q * 512:(q + 1) * 512],
                    start=True, stop=True,
                )
                # relu + f32->bf16 cast, alternating ACT / DVE
                x1s = x1_sb[:, q * 512:(q + 1) * 512]
                if q % 2 == 0:
                    nc.scalar.activation(x1s, ps1[:], relu)
                else:
                    nc.vector.tensor_scalar_max(x1s, ps1[:], 0.0)
            stage = stage_pool.tile([128, SUP // 128, COUT], BF16)
            for j2 in range(SUP // 256):
                ps2 = ps2_pool.tile([128, 2 * COUT], FP32)
                for h in range(2):
                    j = 2 * j2 + h
                    nc.tensor.matmul(
                        ps2[:, h * COUT:(h + 1) * COUT],
                        lhsT=x1_sb[:, j * 128:(j + 1) * 128], rhs=w2t_sb[:],
                        start=True, stop=True,
                    )
                # f32 PSUM -> bf16 SBUF cast; alternate engines to balance
                dst = stage[:, 2 * j2:2 * j2 + 2, :].rearrange("p j c -> p (j c)")
                if j2 % 2 == 0:
                    nc.vector.tensor_copy(dst, ps2[:])
                else:
                    nc.scalar.activation(
                        dst, ps2[:], mybir.ActivationFunctionType.Copy)
            if i < NSUP - 1:
                # 1 MB write; alternate the two HWDGE queues (SP / ACT)
                eng = nc.sync if i % 2 == 0 else nc.scalar
                eng.dma_start(out=x2t_v[i], in_=stage[:])
            else:
                # the last supertile's write gates every gather: split it
                # across both HWDGE queues so it lands sooner
                half = SUP // 256
                nc.sync.dma_start(out=x2t_v[i][:, :half], in_=stage[:, :half])
                nc.scalar.dma_start(out=x2t_v[i][:, half:], in_=stage[:, half:])

        # Phase boundary: gathers must observe every x2t row.
        tc.strict_bb_all_engine_barrier()

    dst_pool = ctx.enter_context(tc.tile_pool(name="dst", bufs=8))
    tail_pool = ctx.enter_context(tc.tile_pool(name="dtail", bufs=4))
    out_pool = ctx.enter_context(tc.tile_pool(name="ostage", bufs=2))
    outT_v = outT.ap().rearrange("(t p) c -> t p c", p=128)  # [47, 128, 256]
    stage2 = None
    mx = mybir.AluOpType.max
    K8 = K // 4           # k-slots per tail sub-call
    NW8 = NW // 4
    for t in range(MT):
        g, r = divmod(t, STG)
        if r == 0:
            n_in_g = min(STG, MT - g * STG)
            stage2 = out_pool.tile([128, STG, COUT], FP32)
        s2 = stage2[:, r, :]
        if t < MT - 1:
            dst = dst_pool.tile([128, K, COUT], BF16)
            nc.gpsimd.dma_gather(
                out_ap=dst[:],
                in_ap=x2t.ap(),
                idxs_ap=idx_sb[:, t * NW:(t + 1) * NW],
                num_idxs=NI,
                num_idxs_reg=NI,
                elem_size=COUT,
                transpose=False,
                single_packet=False,
                queue_num=t % NQ,
            )
            # in-place pairwise max tree over k; flat unit-stride slices
            dv = dst[:].rearrange("p k c -> p (k c)")  # [128, 8192]
            for h in (16, 8, 4, 2):
                w = h * COUT
                nc.vector.tensor_tensor(
                    out=dv[:, :w], in0=dv[:, :w], in1=dv[:, w:2 * w], op=mx
                )
            # final level + f32 upcast
            nc.vector.tensor_tensor(
                out=s2, in0=dv[:, :COUT], in1=dv[:, COUT:2 * COUT], op=mx
            )
        else:
            # last tile: 4x 1024-idx sub-calls (k-slot quarters) across all
            # four queues so the final DMA drain parallelizes instead of
            # trailing one queue by a full 2MB service time. Slicing the
            # wrapped idx list at 64-word boundaries gives sub-call s
            # exactly k in [8s, 8s+8) for all 128 queries.
            dsts = []
            for s in range(4):
                d8 = tail_pool.tile([128, K8, COUT], BF16)
                nc.gpsimd.dma_gather(
                    out_ap=d8[:],
                    in_ap=x2t.ap(),
                    idxs_ap=idx_sb[:, t * NW + s * NW8:t * NW + (s + 1) * NW8],
                    num_idxs=NI // 4,
                    num_idxs_reg=NI // 4,
                    elem_size=COUT,
                    transpose=False,
                    single_packet=False,
                    queue_num=s,
                )
                dsts.append(d8)
            for s in range(4):
                dv = dsts[s][:].rearrange("p k c -> p (k c)")  # [128, 2048]
                for h in (4, 2):
                    w = h * COUT
                    nc.vector.tensor_tensor(
                        out=dv[:, :w], in0=dv[:, :w], in1=dv[:, w:2 * w], op=mx
                    )
                if s == 0:
                    nc.vector.tensor_tensor(
                        out=s2, in0=dv[:, :COUT], in1=dv[:, COUT:2 * COUT], op=mx
                    )
                else:
                    nc.vector.tensor_tensor(
                        out=dv[:, :COUT], in0=dv[:, :COUT],
                        in1=dv[:, COUT:2 * COUT], op=mx
                    )
                    nc.vector.tensor_tensor(out=s2, in0=s2, in1=dv[:, :COUT], op=mx)
        if r == n_in_g - 1:
            eng = nc.sync if g % 2 == 0 else nc.scalar
            eng.dma_start(
                out=outT_v[g * STG:g * STG + n_in_g].rearrange("t p c -> p t c"),
                in_=stage2[:, :n_in_g, :],
            )


def build_program():
    nc = bacc.Bacc("TRN2", target_bir_lowering=False, debug=False,
                   num_swdge_queues=NQ)
    xin = nc.dram_tensor("xin", [CXIN, NPAD], BF16, kind="ExternalInput")
    w1t = nc.dram_tensor("w1t", [CXIN, CMID], BF16, kind="ExternalInput")
    w2t = nc.dram_tensor("w2t", [CMID, COUT], BF16, kind="ExternalInput")
    idxt = nc.dram_tensor(
        "idxt", [128, MT * NW], mybir.dt.int16, kind="ExternalInput"
    )
    outT = nc.dram_tensor("outT", [MPAD, COUT], FP32, kind="ExternalOutput")
    x2t = nc.dram_tensor("x2t", [NPAD, COUT], BF16, kind="Internal")

    from contextlib import ExitStack  # noqa: F811

    with tile.TileContext(nc) as tc:
        with ExitStack() as ctx:
            build_body(ctx, tc, xin, w1t, w2t, idxt, outT, x2t)
    nc.compile()
    return nc


def make_in_map(features_b, xyz_b, idx_b, W1, W2, b1):
    """Build one core's input map from one batch's raw inputs (numpy)."""
    xin = np.zeros((CXIN, NPAD), ml_dtypes.bfloat16)
    xin[:CIN, :N] = features_b
    xin[CIN:CIN + 3, :N] = xyz_b.T
    xin[CIN + 3, :N] = 1.0  # ones row: conv1 bias via matmul
    idxp = np.zeros((MPAD, K), np.int16)
    idxp[:M] = idx_b.astype(np.int16)
    # per call t the index list is i = k*128 + ml -> neighbor[t*128+ml, k]
    # (so index i lands in dst partition i%128 = ml, slot i//128 = k),
    # wrapped into 16 partitions (idx i at [i%16, i//16]) and replicated
    # across the 8 gpsimd cores' partition groups.
    lst = idxp.reshape(MT, 128, K).transpose(0, 2, 1).reshape(MT, NI)
    w = lst.reshape(MT, NW, 16).transpose(0, 2, 1)      # [MT, 16, NW]
    w = np.tile(w, (1, 8, 1))                           # [MT, 128, NW]
    idxt = np.ascontiguousarray(
        w.transpose(1, 0, 2).reshape(128, MT * NW).astype(np.int16)
    )
    w1e = np.concatenate(
        [W1.astype(np.float32), b1.astype(np.float32)[:, None]], axis=1
    )  # [CMID, CXIN]
    return {
        "xin": xin,
        "w1t": np.ascontiguousarray(w1e.T.astype(ml_dtypes.bfloat16)),
        "w2t": np.ascontiguousarray(W2.T.astype(ml_dtypes.bfloat16)),
        "idxt": idxt,
    }


def postprocess(outT_np, b2):
    """[MPAD, 256] device result -> [256, 6000] final (+b2, relu, T)."""
    o = outT_np[:M].astype(np.float32) + b2.astype(np.float32)[None, :]
    np.maximum(o, 0.0, out=o)
    return np.ascontiguousarray(o.T)


def run(inputs, trace=False, **spmd_kwargs):
    """Run on 8 NeuronCores; returns (out [8,256,6000] f32, BassKernelResults)."""
    features = np.asarray(inputs["features"], np.float32)
    support_xyz = np.asarray(inputs["support_xyz"], np.float32)
    neighbor_idx = np.asarray(inputs["neighbor_idx"])
    W1 = np.asarray(inputs["W1"], np.float32)
    W2 = np.asarray(inputs["W2"], np.float32)
    b1 = np.asarray(inputs["b1"], np.float32)
    b2 = np.asarray(inputs["b2"], np.float32)

    if "nc" not in _CACHE:
        _CACHE["nc"] = build_program()
    nc = _CACHE["nc"]

    in_maps = [
        make_in_map(features[b], support_xyz[b], neighbor_idx[b], W1, W2, b1)
        for b in range(B)
    ]
    res = run_bass_kernel_spmd(
        nc, in_maps, core_ids=list(range(B)), trace=trace, **spmd_kwargs
    )
    out = np.stack(
        [postprocess(res.results[b]["outT"], b2) for b in range(B)]
    ).astype(np.float32)
    return out, res


def kernel(query_xyz, support_xyz, features, neighbor_idx, W1, b1, W2, b2,
           **unused):
    del query_xyz  # neighborhoods are precomputed; query coords unused
    out, _ = run(
        dict(
            support_xyz=support_xyz,
            features=features,
            neighbor_idx=neighbor_idx,
            W1=W1,
            b1=b1,
            W2=W2,
            b2=b2,
        )
    )
    return out


if __name__ == "__main__":
    nc = build_program()
    print("program built ok")


# revision 29
# speedup vs baseline: 1.0186x; 1.0186x over previous
"""Trainium2 Bass kernel for DelayedAgg GNN message passing.

Per batch b (one NeuronCore per batch, B=8 across 8 cores):
    xin  = concat(features[b], support_xyz[b].T, ones)    # [68, N] bf16
    x1   = relu(W1' @ xin)                                # [128, N] (b1 folded
                                                          #  into W1' col 67)
    x2nb = W2 @ x1                                        # [256, N] (NO bias)
    out[c, m] = relu(max_k x2nb[c, idx[m,k]] + b2[c])
bias2 + relu2 commute out of the neighbor max, so the device computes only
max_k of bias-free conv2 rows; +b2/relu/transpose run on the host over the
small [6016, 256] result.

Measured 947-1058us baseline -> 573us on this kernel. Changes that got it:
  - All conv matmuls in bf16 (PE fp32 runs at 1/4 rate; fp32 convs made
    phase 1 ~330us wall; bf16 phase 1 is ~100us).  b1 folded into the matmul
    via a constant ones row (activation bias can't vary along the free dim
    in the transposed conv).  rel_err 0.002 -> 0.0049, still << 2e-2.
  - conv2 pairs two 128-point chunks into one [128, 512] PSUM tile so the
    PSUM->SBUF bf16 casts are half as many, alternating DVE/ACT; conv1
    relu+cast alternates ACT activation / DVE tensor_scalar_max.
  - dst pool bufs 4 -> 8 (phase-1 pools released first) removes
    gather-to-tree buffer-reuse stalls: descgen drops from 14.4us to
    ~10us per 4096-row call (the baseline's extra 4us was WAW sem waits
    inside the gather's engine slot, not ucode time).
  - 3MB idx table DMA moved to the ACT HWDGE queue so the first conv
    supertile's xin load isn't queued behind it on sync (~12us).
  - last supertile's x2t write split across both HWDGE queues (it gates
    every gather); last m-tile gathered as 4x 1024-idx sub-calls across
    all 4 queues so the final 2MB drain parallelizes instead of trailing
    one queue.
  - NEVER make a matmul output PSUM tile span >1 bank (2KB/part): a
    [128, 1024] f32 ps2 tile put the PE on a ~100x slow path (7.8ms).
Phase 2 is descgen-bound: InstDMAGatherAnt descriptor generation runs on one
Q7 core pair at ~2.5ns/row-descriptor and each call occupies the whole gpsimd
engine (queue_num only picks the SDMA ring), so 192k gathered rows floor at
~460-490us.  Dead ends verified on HW this session: prepare_only descgen
cannot hide under phase 1 (SWDGE ring carveout ~1024 desc/queue, one 4096-row
prep overflows it and deadlocks -> NRT INTERNAL error); indirect_dma_start
unrolls to per-index DMAs (249us per 4096 rows, and wrong layout); SBUF-source
and DRAM transpose-mode gathers (256B rows) measure no faster than the plain
512B gather (descgen is the wall, not SDMA); plain 256B-row gather is 4x
SLOWER (59.9us/call anomaly); activity-throttle (50% duty, active ~90% of
phase 2) does NOT slow the Q7 descgen path measurably.
Pitfalls baked in: single_packet=False (HW caps packets at ~64 descriptors);
cce_op=max rejected (CCE only does add).
"""

import os
import sys

import numpy as np

try:
    import concourse.bass as bass  # noqa: F401
except ImportError:  # pragma: no cover - container default path
    sys.path.insert(0, "/opt/trn_rl_repo")

import concourse.bass as bass
import concourse.bacc as bacc
import concourse.tile as tile
from concourse import mybir
from concourse.bass_utils import run_bass_kernel_spmd

import ml_dtypes

# Problem shapes (hardcoded per spec nn_DelayedAgg_76690936037739)
B = 8
N = 24000
M = 6000
K = 32
CIN = 64
CMID = 128
COUT = 256
CXIN = CIN + 3 + 1    # features + xyz + ones row (bias fold)

SUP = 2048            # support points per phase-1 supertile
NPAD = 24576          # 12 * 2048 = 192 * 128
NSUP = NPAD // SUP    # 12
MT = 47               # m-tiles of 128 queries
MPAD = MT * 128       # 6016
STG = 8               # m-tiles per output staging buffer
GRP = (MT + STG - 1) // STG   # 6 output groups
OUTROWS = GRP * 128 * STG     # 6144 (group-padded, partition-major layout)
T = 128               # queries per gather call
NI = K * T            # 4096 indices per call
NW = NI // 16         # idx int16 words per partition per call
NQ = 4                # SWDGE queues for gather descgen

FP32 = mybir.dt.float32
BF16 = mybir.dt.bfloat16

_CACHE = {}


def build_body(ctx, tc, xin, w1t, w2t, idxt, outT, x2t):
    nc = tc.nc

    singles = ctx.enter_context(tc.tile_pool(name="singles", bufs=1))
    w1t_sb = singles.tile([CXIN, CMID], BF16)
    nc.sync.dma_start(out=w1t_sb[:], in_=w1t.ap())
    w2t_sb = singles.tile([CMID, COUT], BF16)
    nc.sync.dma_start(out=w2t_sb[:], in_=w2t.ap())
    # 3MB idx load goes on the gpsimd SWDGE queue: that engine is idle all
    # of phase 1 (and is the eventual consumer), so neither the sync-queue
    # xin loads nor the ACT-queue relus ever queue behind it.
    idx_sb = singles.tile([128, MT * NW], mybir.dt.int16)
    nc.gpsimd.dma_start(out=idx_sb[:], in_=idxt.ap())

    from contextlib import ExitStack

    with ExitStack() as p1:
        xin_pool = p1.enter_context(tc.tile_pool(name="xin", bufs=3))
        ps1_pool = p1.enter_context(tc.tile_pool(name="ps1", bufs=2, space="PSUM"))
        x1_pool = p1.enter_context(tc.tile_pool(name="x1", bufs=3))
        ps2_pool = p1.enter_context(tc.tile_pool(name="ps2", bufs=6, space="PSUM"))
        stage_pool = p1.enter_context(tc.tile_pool(name="stage", bufs=3))

        xin_ap = xin.ap()                                   # [68, NPAD]
        # x2t row i*2048 + p*16 + j  <-  stage[p, j, :]: partition-major so
        # each partition writes one contiguous 8KB run -> 128 HWDGE
        # descriptors per 1MB write instead of 2048 (the 512B row-interleaved
        # layout made the write ~11us of HWDGE descriptor service and
        # saturated the sync/ACT queues that also carry xin loads).  Host
        # remaps gather indices to match (free).
        x2t_v = x2t.ap().rearrange("(i p t) c -> i p t c", p=128, t=SUP // 128)

        relu = mybir.ActivationFunctionType.Relu
        mx0 = mybir.AluOpType.max
        for i in range(NSUP):
            xin_sb = xin_pool.tile([CXIN, SUP], BF16)
            nc.sync.dma_start(out=xin_sb[:], in_=xin_ap[:, i * SUP:(i + 1) * SUP])
            x1_sb = x1_pool.tile([CMID, SUP], BF16)
            for q in range(SUP // 512):
                ps1 = ps1_pool.tile([CMID, 512], FP32)
                nc.tensor.matmul(
                    ps1[:], lhsT=w1t_sb[:], rhs=xin_sb[:, q * 512:(q + 1) * 512],
                    start=True, stop=True,
                )
                # relu + f32->bf16 cast, alternating ACT / DVE
                x1s = x1_sb[:, q * 512:(q + 1) * 512]
                if q % 2 == 0:
                    nc.scalar.activation(x1s, ps1[:], relu)
                else:
                    nc.vector.tensor_scalar_max(x1s, ps1[:], 0.0)
            stage = stage_pool.tile([128, SUP // 128, COUT], BF16)
            for j2 in range(SUP // 256):
                ps2 = ps2_pool.tile([128, 2 * COUT], FP32)
                for h in range(2):
                    j = 2 * j2 + h
                    nc.tensor.matmul(
                        ps2[:, h * COUT:(h + 1) * COUT],
                        lhsT=x1_sb[:, j * 128:(j + 1) * 128], rhs=w2t_sb[:],
                        start=True, stop=True,
                    )
                # f32 PSUM -> bf16 SBUF cast; alternate engines to balance
                dst = stage[:, 2 * j2:2 * j2 + 2, :].rearrange("p j c -> p (j c)")
                if j2 % 2 == 0:
                    nc.vector.tensor_copy(dst, ps2[:])
                else:
                    nc.scalar.activation(
                        dst, ps2[:], mybir.ActivationFunctionType.Copy)
            if i < NSUP - 1:
                # 1 MB write; alternate the two HWDGE queues (SP / ACT)
                eng = nc.sync if i % 2 == 0 else nc.scalar
                eng.dma_start(out=x2t_v[i], in_=stage[:])
            else:
                # the last supertile's write gates every gather: split it
                # across both HWDGE queues so it lands sooner
                half = SUP // 256
                nc.sync.dma_start(out=x2t_v[i][:, :half], in_=stage[:, :half])
                nc.scalar.dma_start(out=x2t_v[i][:, half:], in_=stage[:, half:])

        # Phase boundary: gathers must observe every x2t row.
        tc.strict_bb_all_engine_barrier()

    dst_pool = ctx.enter_context(tc.tile_pool(name="dst", bufs=8))
    tail_pool = ctx.enter_context(tc.tile_pool(name="dtail", bufs=4))
    out_pool = ctx.enter_context(tc.tile_pool(name="ostage", bufs=2))
    # outT row g*1024 + p*8 + t: partition-major per group (contiguous 8KB
    # per partition per write); host unscrambles.
    outT_v = outT.ap().rearrange("(g p t) c -> g p t c", p=128, t=STG)
    stage2 = None
    mx = mybir.AluOpType.max
    K8 = K // 4           # k-slots per tail sub-call
    NW8 = NW // 4
    for t in range(MT):
        g, r = divmod(t, STG)
        if r == 0:
            n_in_g = min(STG, MT - g * STG)
            stage2 = out_pool.tile([128, STG, COUT], FP32)
        s2 = stage2[:, r, :]
        if t < MT - 1:
            dst = dst_pool.tile([128, K, COUT], BF16)
            nc.gpsimd.dma_gather(
                out_ap=dst[:],
                in_ap=x2t.ap(),
                idxs_ap=idx_sb[:, t * NW:(t + 1) * NW],
                num_idxs=NI,
                num_idxs_reg=NI,
                elem_size=COUT,
                transpose=False,
                single_packet=False,
                queue_num=t % NQ,
            )
            # in-place pairwise max tree over k; flat unit-stride slices
            dv = dst[:].rearrange("p k c -> p (k c)")  # [128, 8192]
            for h in (16, 8, 4, 2):
                w = h * COUT
                nc.vector.tensor_tensor(
                    out=dv[:, :w], in0=dv[:, :w], in1=dv[:, w:2 * w], op=mx
                )
            # final level + f32 upcast
            nc.vector.tensor_tensor(
                out=s2, in0=dv[:, :COUT], in1=dv[:, COUT:2 * COUT], op=mx
            )
        else:
            # last tile: 4x 1024-idx sub-calls (k-slot quarters) across all
            # four queues so the final DMA drain parallelizes instead of
            # trailing one queue by a full 2MB service time. Slicing the
            # wrapped idx list at 64-word boundaries gives sub-call s
            # exactly k in [8s, 8s+8) for all 128 queries.
            dsts = []
            for s in range(4):
                d8 = tail_pool.tile([128, K8, COUT], BF16)
                nc.gpsimd.dma_gather(
                    out_ap=d8[:],
                    in_ap=x2t.ap(),
                    idxs_ap=idx_sb[:, t * NW + s * NW8:t * NW + (s + 1) * NW8],
                    num_idxs=NI // 4,
                    num_idxs_reg=NI // 4,
                    elem_size=COUT,
                    transpose=False,
                    single_packet=False,
                    queue_num=s,
                )
                dsts.append(d8)
            for s in range(4):
                dv = dsts[s][:].rearrange("p k c -> p (k c)")  # [128, 2048]
                for h in (4, 2):
                    w = h * COUT
                    nc.vector.tensor_tensor(
                        out=dv[:, :w], in0=dv[:, :w], in1=dv[:, w:2 * w], op=mx
                    )
                if s == 0:
                    nc.vector.tensor_tensor(
                        out=s2, in0=dv[:, :COUT], in1=dv[:, COUT:2 * COUT], op=mx
                    )
                else:
                    nc.vector.tensor_tensor(
                        out=dv[:, :COUT], in0=dv[:, :COUT],
                        in1=dv[:, COUT:2 * COUT], op=mx
                    )
                    nc.vector.tensor_tensor(out=s2, in0=s2, in1=dv[:, :COUT], op=mx)
        if r == n_in_g - 1:
            eng = nc.sync if g % 2 == 0 else nc.scalar
            eng.dma_start(
                out=outT_v[g][:, :n_in_g, :],
                in_=stage2[:, :n_in_g, :],
            )


def build_program():
    nc = bacc.Bacc("TRN2", target_bir_lowering=False, debug=False,
                   num_swdge_queues=NQ)
    xin = nc.dram_tensor("xin", [CXIN, NPAD], BF16, kind="ExternalInput")
    w1t = nc.dram_tensor("w1t", [CXIN, CMID], BF16, kind="ExternalInput")
    w2t = nc.dram_tensor("w2t", [CMID, COUT], BF16, kind="ExternalInput")
    idxt = nc.dram_tensor(
        "idxt", [128, MT * NW], mybir.dt.int16, kind="ExternalInput"
    )
    outT = nc.dram_tensor("outT", [OUTROWS, COUT], FP32, kind="ExternalOutput")
    x2t = nc.dram_tensor("x2t", [NPAD, COUT], BF16, kind="Internal")

    from contextlib import ExitStack  # noqa: F811

    with tile.TileContext(nc) as tc:
        with ExitStack() as ctx:
            build_body(ctx, tc, xin, w1t, w2t, idxt, outT, x2t)
    nc.compile()
    return nc


def make_in_map(features_b, xyz_b, idx_b, W1, W2, b1):
    """Build one core's input map from one batch's raw inputs (numpy)."""
    xin = np.zeros((CXIN, NPAD), ml_dtypes.bfloat16)
    xin[:CIN, :N] = features_b
    xin[CIN:CIN + 3, :N] = xyz_b.T
    xin[CIN + 3, :N] = 1.0  # ones row: conv1 bias via matmul
    # remap point n -> x2t row for the partition-major supertile layout:
    # row = (n//2048)*2048 + (n%2048 % 128)*16 + (n%2048)//128
    nb = idx_b.astype(np.int32)
    rem = nb % SUP
    row = (nb - rem) + (rem % 128) * (SUP // 128) + rem // 128
    idxp = np.zeros((MPAD, K), np.int16)
    idxp[:M] = row.astype(np.int16)
    # per call t the index list is i = k*128 + ml -> neighbor[t*128+ml, k]
    # (so index i lands in dst partition i%128 = ml, slot i//128 = k),
    # wrapped into 16 partitions (idx i at [i%16, i//16]) and replicated
    # across the 8 gpsimd cores' partition groups.
    lst = idxp.reshape(MT, 128, K).transpose(0, 2, 1).reshape(MT, NI)
    w = lst.reshape(MT, NW, 16).transpose(0, 2, 1)      # [MT, 16, NW]
    w = np.tile(w, (1, 8, 1))                           # [MT, 128, NW]
    idxt = np.ascontiguousarray(
        w.transpose(1, 0, 2).reshape(128, MT * NW).astype(np.int16)
    )
    w1e = np.concatenate(
        [W1.astype(np.float32), b1.astype(np.float32)[:, None]], axis=1
    )  # [CMID, CXIN]
    return {
        "xin": xin,
        "w1t": np.ascontiguousarray(w1e.T.astype(ml_dtypes.bfloat16)),
        "w2t": np.ascontiguousarray(W2.T.astype(ml_dtypes.bfloat16)),
        "idxt": idxt,
    }


def postprocess(outT_np, b2):
    """[OUTROWS, 256] device result -> [256, 6000] final (+b2, relu, T).

    Device row g*1024 + p*8 + t holds query (g*8 + t)*128 + p."""
    o = (outT_np.reshape(GRP, 128, STG, COUT).transpose(0, 2, 1, 3)
         .reshape(OUTROWS, COUT)[:M].astype(np.float32))
    o += b2.astype(np.float32)[None, :]
    np.maximum(o, 0.0, out=o)
    return np.ascontiguousarray(o.T)


def run(inputs, trace=False, **spmd_kwargs):
    """Run on 8 NeuronCores; returns (out [8,256,6000] f32, BassKernelResults)."""
    features = np.asarray(inputs["features"], np.float32)
    support_xyz = np.asarray(inputs["support_xyz"], np.float32)
    neighbor_idx = np.asarray(inputs["neighbor_idx"])
    W1 = np.asarray(inputs["W1"], np.float32)
    W2 = np.asarray(inputs["W2"], np.float32)
    b1 = np.asarray(inputs["b1"], np.float32)
    b2 = np.asarray(inputs["b2"], np.float32)

    if "nc" not in _CACHE:
        _CACHE["nc"] = build_program()
    nc = _CACHE["nc"]

    in_maps = [
        make_in_map(features[b], support_xyz[b], neighbor_idx[b], W1, W2, b1)
        for b in range(B)
    ]
    res = run_bass_kernel_spmd(
        nc, in_maps, core_ids=list(range(B)), trace=trace, **spmd_kwargs
    )
    out = np.stack(
        [postprocess(res.results[b]["outT"], b2) for b in range(B)]
    ).astype(np.float32)
    return out, res


def kernel(query_xyz, support_xyz, features, neighbor_idx, W1, b1, W2, b2,
           **unused):
    del query_xyz  # neighborhoods are precomputed; query coords unused
    out, _ = run(
        dict(
            support_xyz=support_xyz,
            features=features,
            neighbor_idx=neighbor_idx,
            W1=W1,
            b1=b1,
            W2=W2,
            b2=b2,
        )
    )
    return out


if __name__ == "__main__":
    nc = build_program()
    print("program built ok")
